# revision 4
# baseline (speedup 1.0000x reference)
"""Trainium2 Bass kernel for nn_BiquadFilter.

Math: the reference builds, per batch, an 8192-tap FIR from 6 cascaded
biquads (frequency sampling: rfft of 3-tap coeff arrays -> cascade product
-> irfft), then linearly convolves each [C=2, L=524288] signal with it
(causal, truncated to L).

Device implementation (one batch per NeuronCore, 8 cores):
 1. tanh-activations of the feedback coefficients, broadcast to 128
    partitions via a ones-matmul.
 2. Frequency response H[f] on a [u=128, j=33] grid (f = u + 128 j) via
    vector ops with host-provided cos/sin tables.
 3. irfft(8192) as a 3-step factorization (contract j with a 33x128 DFT
    basis; pointwise twiddle; contract u with a 128x64 basis), giving
    fir[p + 128 q] laid out [q=64, p=128]; stored to a DRAM scratch with
    128-zero margins.
 4. 65 Hankel-shaped stationaries hk_j[v, p] = fir[128(j-1) + 1 + p + v]
    loaded back in one overlapping-stride DMA (cast to float32r).
 5. Convolution as 2 x 8 x 65 accumulating matmuls in float32r:
    y[p, 128 f] block-tiles of [128, 512] in PSUM; the input signal is
    host-relaid-out as xr[v, c, blk] = x[c, 128 blk + 127 - v] with 64
    zero pad blocks per channel (so the stationary needs only positive
    strides).
"""

import numpy as np

FIR_LEN = 8192
L = 524288
C = 2
B = 8
K = 6
NB = L // 128            # 4096 blocks per channel
NPAD = 64                # causal zero-pad blocks
NJ = 33                  # f chunks (33*128 = 4224 >= 4097)
NQ = 64                  # fir rows (64*128 = 8192)
NHK = 65                 # conv stationaries
FT = NB // 512           # free tiles per channel (8)

_CACHE = {}


def _build_constants():
    f = np.arange(NJ * 128)
    w = np.zeros(NJ * 128, np.float64)
    w[0] = 1.0
    w[4096] = 1.0
    w[1:4096] = 2.0
    w /= FIR_LEN
    th = 2.0 * np.pi * f / FIR_LEN
    c1 = np.cos(th)
    s1 = -np.sin(th)
    c2 = np.cos(2 * th)
    s2 = -np.sin(2 * th)
    for a in (c1, s1, c2, s2):
        a[4097:] = 0.0
    w[4097:] = 0.0

    def t(a):
        return np.ascontiguousarray(a.reshape(NJ, 128).T.astype(np.float32))

    u = np.arange(128)
    p = np.arange(128)
    j = np.arange(NJ)
    q = np.arange(NQ)
    Are = np.cos(2 * np.pi * np.outer(u, p) / FIR_LEN).astype(np.float32)
    Aim = np.sin(2 * np.pi * np.outer(u, p) / FIR_LEN).astype(np.float32)
    Bre = np.cos(2 * np.pi * np.outer(j, p) / 64).astype(np.float32)
    Bim = np.sin(2 * np.pi * np.outer(j, p) / 64).astype(np.float32)
    Cre = np.cos(2 * np.pi * np.outer(u, q) / 64).astype(np.float32)
    Cim = np.sin(2 * np.pi * np.outer(u, q) / 64).astype(np.float32)
    return {
        "c1": t(c1), "s1": t(s1), "c2": t(c2), "s2": t(s2), "wt": t(w),
        "Are": Are, "Aim": Aim,
        "Bre": np.ascontiguousarray(Bre), "Bim": np.ascontiguousarray(Bim),
        "Bimn": np.ascontiguousarray(-Bim),
        "Cre": np.ascontiguousarray(Cre), "Cimn": np.ascontiguousarray(-Cim),
        "ones": np.ones((1, 128), np.float32),
        "ident": np.eye(128, dtype=np.float32),
    }


def _build_program():
    import concourse.bass as bass
    import concourse.bacc as bacc
    import concourse.tile as tile
    from concourse import mybir

    F32 = mybir.dt.float32
    F32R = mybir.dt.float32r
    ACT = mybir.ActivationFunctionType

    nc = bacc.Bacc("TRN2", target_bir_lowering=False, debug=False,
                   enable_asserts=False)

    xt_d = nc.dram_tensor("xt", [128, C * (NPAD + NB)], F32,
                          kind="ExternalInput")
    coef_d = nc.dram_tensor("coef", [1, 30], F32, kind="ExternalInput")
    tabs_d = {n: nc.dram_tensor(n, [128, NJ], F32, kind="ExternalInput")
              for n in ("c1", "s1", "c2", "s2", "wt")}
    Are_d = nc.dram_tensor("Are", [128, 128], F32, kind="ExternalInput")
    Aim_d = nc.dram_tensor("Aim", [128, 128], F32, kind="ExternalInput")
    Bre_d = nc.dram_tensor("Bre", [NJ, 128], F32, kind="ExternalInput")
    Bim_d = nc.dram_tensor("Bim", [NJ, 128], F32, kind="ExternalInput")
    Bimn_d = nc.dram_tensor("Bimn", [NJ, 128], F32, kind="ExternalInput")
    Cre_d = nc.dram_tensor("Cre", [128, NQ], F32, kind="ExternalInput")
    Cimn_d = nc.dram_tensor("Cimn", [128, NQ], F32, kind="ExternalInput")
    ones_d = nc.dram_tensor("ones", [1, 128], F32, kind="ExternalInput")
    ident_d = nc.dram_tensor("ident", [128, 128], F32, kind="ExternalInput")

    yt_d = nc.dram_tensor("yt", [128, C, NB], F32, kind="ExternalOutput")
    P_d = nc.dram_tensor("P", [FIR_LEN + 256], F32, kind="ExternalOutput")

    with tile.TileContext(nc) as tc:
        with (
            tc.tile_pool(name="const", bufs=1) as cpool,
            tc.tile_pool(name="big", bufs=1) as big,
            tc.tile_pool(name="work", bufs=2) as work,
            tc.tile_pool(name="out", bufs=3) as outp,
        ):
            # ---- big input load + f32r cast ----
            xt32 = big.tile([128, C * (NPAD + NB)], F32)
            nc.sync.dma_start(xt32[:], xt_d.ap())
            xr = big.tile([128, C * (NPAD + NB)], F32R)
            half = C * (NPAD + NB) // 2
            nc.vector.tensor_copy(xr[:, 0:half], xt32[:, 0:half])
            nc.vector.tensor_copy(xr[:, half:], xt32[:, half:])

            # ---- constants ----
            tabs = {}
            for n, d in tabs_d.items():
                tabs[n] = cpool.tile([128, NJ], F32, tag=n, name=n)
                nc.sync.dma_start(tabs[n][:], d.ap())
            Are = cpool.tile([128, 128], F32, tag="Are")
            nc.sync.dma_start(Are[:], Are_d.ap())
            Aim = cpool.tile([128, 128], F32, tag="Aim")
            nc.sync.dma_start(Aim[:], Aim_d.ap())
            Bre = cpool.tile([NJ, 128], F32, tag="Bre")
            nc.sync.dma_start(Bre[:], Bre_d.ap())
            Bim = cpool.tile([NJ, 128], F32, tag="Bim")
            nc.sync.dma_start(Bim[:], Bim_d.ap())
            Bimn = cpool.tile([NJ, 128], F32, tag="Bimn")
            nc.sync.dma_start(Bimn[:], Bimn_d.ap())
            Cre = cpool.tile([128, NQ], F32, tag="Cre")
            nc.sync.dma_start(Cre[:], Cre_d.ap())
            Cimn = cpool.tile([128, NQ], F32, tag="Cimn")
            nc.sync.dma_start(Cimn[:], Cimn_d.ap())
            ones = cpool.tile([1, 128], F32, tag="ones")
            nc.sync.dma_start(ones[:], ones_d.ap())
            ident = cpool.tile([128, 128], F32, tag="ident")
            nc.sync.dma_start(ident[:], ident_d.ap())

            # ---- coefficient activations on [1, n] tiles ----
            sc = cpool.tile([1, 30], F32, tag="sc")
            nc.sync.dma_start(sc[:], coef_d.ap())
            th = cpool.tile([1, 12], F32, tag="th")
            nc.scalar.activation(th[:], sc[:, 18:30], ACT.Tanh)
            scal = cpool.tile([1, 30], F32, tag="scal")
            nc.vector.tensor_copy(scal[:, 0:18], sc[:, 0:18])
            nc.scalar.mul(scal[:, 18:24], th[:, 0:6], 2.0)          # A1
            ab = cpool.tile([1, 6], F32, tag="ab")
            nc.scalar.activation(ab[:], scal[:, 18:24], ACT.Abs)    # |A1|
            tm = cpool.tile([1, 6], F32, tag="tm")
            nc.vector.tensor_mul(tm[:], ab[:], th[:, 6:12])         # |A1| t2
            x1 = cpool.tile([1, 6], F32, tag="x1")
            nc.scalar.mul(x1[:], tm[:], -0.5)
            x2 = cpool.tile([1, 6], F32, tag="x2")
            nc.scalar.mul(x2[:], ab[:], 0.5)
            x3 = cpool.tile([1, 6], F32, tag="x3")
            nc.vector.tensor_add(x3[:], th[:, 6:12], x1[:])
            nc.vector.tensor_add(scal[:, 24:30], x3[:], x2[:])      # A2

            with tc.tile_pool(name="pps", bufs=1, space="PSUM") as pps:
                # broadcast the 30 scalars to all partitions
                bc_ps = pps.tile([128, 30], F32, tag="bc")
                nc.tensor.matmul(bc_ps[:], ones[:], scal[:],
                                 start=True, stop=True)
                bc = cpool.tile([128, 30], F32, tag="bc_sb")
                nc.vector.tensor_copy(bc[:], bc_ps[:])

                # ---- H[f] on [128, 33] ----
                c1, s1, c2, s2 = tabs["c1"], tabs["s1"], tabs["c2"], tabs["s2"]

                def cplx_scaled(b1k, b2k, basis_a, basis_b, extra, otag):
                    """out = basis_a*b1k + basis_b*b2k (+ extra)."""
                    t1 = work.tile([128, NJ], F32, tag="t1", name="t1")
                    nc.vector.tensor_scalar_mul(t1[:], basis_a[:], b1k)
                    t2 = work.tile([128, NJ], F32, tag="t2", name="t2")
                    if extra is None:
                        nc.vector.tensor_scalar_mul(t2[:], basis_b[:], b2k)
                    else:
                        nc.vector.tensor_scalar(t2[:], basis_b[:], b2k, extra,
                                                mybir.AluOpType.mult,
                                                mybir.AluOpType.add)
                    o = work.tile([128, NJ], F32, tag=otag, name=otag)
                    nc.vector.tensor_add(o[:], t1[:], t2[:])
                    return o

                numre = numim = denre = denim = None
                for k in range(K):
                    b0k = bc[:, k:k + 1]
                    b1k = bc[:, 6 + k:7 + k]
                    b2k = bc[:, 12 + k:13 + k]
                    a1k = bc[:, 18 + k:19 + k]
                    a2k = bc[:, 24 + k:25 + k]
                    bfre = cplx_scaled(b1k, b2k, c1, c2, b0k, "bf_re")
                    bfim = cplx_scaled(b1k, b2k, s1, s2, None, "bf_im")
                    afre = cplx_scaled(a1k, a2k, c1, c2, 1.0, "af_re")
                    afim = cplx_scaled(a1k, a2k, s1, s2, None, "af_im")
                    if k == 0:
                        numre, numim, denre, denim = bfre, bfim, afre, afim
                    else:
                        def cmul(are, aim, bre, bim, tagp):
                            t1 = work.tile([128, NJ], F32, tag="m1")
                            nc.vector.tensor_mul(t1[:], are[:], bre[:])
                            t2 = work.tile([128, NJ], F32, tag="m2")
                            nc.vector.tensor_mul(t2[:], aim[:], bim[:])
                            orr = work.tile([128, NJ], F32, tag=tagp + "re")
                            nc.vector.tensor_sub(orr[:], t1[:], t2[:])
                            nc.vector.tensor_mul(t1[:], are[:], bim[:])
                            nc.vector.tensor_mul(t2[:], aim[:], bre[:])
                            oi = work.tile([128, NJ], F32, tag=tagp + "im")
                            nc.vector.tensor_add(oi[:], t1[:], t2[:])
                            return orr, oi
                        numre, numim = cmul(numre, numim, bfre, bfim, "num")
                        denre, denim = cmul(denre, denim, afre, afim, "den")

                # H = num * conj(den) / |den|^2, then * w
                d1 = work.tile([128, NJ], F32, tag="d1")
                nc.vector.tensor_mul(d1[:], denre[:], denre[:])
                d2 = work.tile([128, NJ], F32, tag="d2")
                nc.vector.tensor_mul(d2[:], denim[:], denim[:])
                dd = work.tile([128, NJ], F32, tag="dd")
                nc.vector.tensor_add(dd[:], d1[:], d2[:])
                rcp = work.tile([128, NJ], F32, tag="rcp")
                nc.vector.reciprocal(rcp[:], dd[:])

                def hpart(t1in, t2in, sub, tagp):
                    t1 = work.tile([128, NJ], F32, tag="h1")
                    nc.vector.tensor_mul(t1[:], t1in[0][:], t1in[1][:])
                    t2 = work.tile([128, NJ], F32, tag="h2")
                    nc.vector.tensor_mul(t2[:], t2in[0][:], t2in[1][:])
                    hs = work.tile([128, NJ], F32, tag=tagp + "s")
                    if sub:
                        nc.vector.tensor_sub(hs[:], t1[:], t2[:])
                    else:
                        nc.vector.tensor_add(hs[:], t1[:], t2[:])
                    hr = work.tile([128, NJ], F32, tag=tagp + "r")
                    nc.vector.tensor_mul(hr[:], hs[:], rcp[:])
                    o = work.tile([128, NJ], F32, tag=tagp)
                    nc.vector.tensor_mul(o[:], hr[:], tabs["wt"][:])
                    return o

                wHre = hpart((numre, denre), (numim, denim), False, "wHre")
                wHim = hpart((numim, denre), (numre, denim), True, "wHim")

                # ---- transpose [128, 33] -> [33, 128] ----
                whreT_ps = pps.tile([NJ, 128], F32, tag="whreT")
                nc.tensor.transpose(whreT_ps[:], wHre[:], ident[:])
                whreT = work.tile([NJ, 128], F32, tag="whreTs")
                nc.vector.tensor_copy(whreT[:], whreT_ps[:])
                whimT_ps = pps.tile([NJ, 128], F32, tag="whimT")
                nc.tensor.transpose(whimT_ps[:], wHim[:], ident[:])
                whimT = work.tile([NJ, 128], F32, tag="whimTs")
                nc.vector.tensor_copy(whimT[:], whimT_ps[:])

                # ---- stage 1: T[u,p] = sum_j wH[u,j] B[j,p] ----
                tre_ps = pps.tile([128, 128], F32, tag="tre")
                nc.tensor.matmul(tre_ps[:], whreT[:], Bre[:],
                                 start=True, stop=False)
                nc.tensor.matmul(tre_ps[:], whimT[:], Bimn[:],
                                 start=False, stop=True)
                tim_ps = pps.tile([128, 128], F32, tag="tim")
                nc.tensor.matmul(tim_ps[:], whreT[:], Bim[:],
                                 start=True, stop=False)
                nc.tensor.matmul(tim_ps[:], whimT[:], Bre[:],
                                 start=False, stop=True)
                tre = work.tile([128, 128], F32, tag="tres")
                nc.vector.tensor_copy(tre[:], tre_ps[:])
                tim = work.tile([128, 128], F32, tag="tims")
                nc.vector.tensor_copy(tim[:], tim_ps[:])

                # ---- U = A (.) T ----
                u1 = work.tile([128, 128], F32, tag="u1")
                nc.vector.tensor_mul(u1[:], Are[:], tre[:])
                u2 = work.tile([128, 128], F32, tag="u2")
                nc.vector.tensor_mul(u2[:], Aim[:], tim[:])
                ure = work.tile([128, 128], F32, tag="ure")
                nc.vector.tensor_sub(ure[:], u1[:], u2[:])
                nc.vector.tensor_mul(u1[:], Are[:], tim[:])
                nc.vector.tensor_mul(u2[:], Aim[:], tre[:])
                uim = work.tile([128, 128], F32, tag="uim")
                nc.vector.tensor_add(uim[:], u1[:], u2[:])

                # ---- stage 2: fir[q,p] = sum_u Cre U_re - Cim U_im ----
                fir_ps = pps.tile([NQ, 128], F32, tag="fir")
                nc.tensor.matmul(fir_ps[:], Cre[:], ure[:],
                                 start=True, stop=False)
                nc.tensor.matmul(fir_ps[:], Cimn[:], uim[:],
                                 start=False, stop=True)
                fir_sb = work.tile([NQ, 128], F32, tag="firs")
                nc.vector.tensor_copy(fir_sb[:], fir_ps[:])
                dst = bass.AP(tensor=P_d, offset=128, ap=[[128, NQ], [1, 128]])
                nc.sync.dma_start(dst, fir_sb[:])

            # ---- Hankel stationaries (cast to f32r during DMA) ----
            hk = big.tile([128, NHK * 128], F32R)
            src = bass.AP(tensor=P_d, offset=1,
                          ap=[[1, 128], [128, NHK], [1, 128]])
            nc.gpsimd.dma_start(
                hk[:].rearrange("v (j p) -> v j p", j=NHK), src)

            # ---- convolution ----
            with tc.tile_pool(name="ypsum", bufs=1, space="PSUM") as yps_pool:
                for c in range(C):
                    yps = [yps_pool.tile([128, 512], F32, tag=f"y{ft}", name=f"y{ft}")
                           for ft in range(FT)]
                    for j in range(NHK):
                        lhs = hk[:, j * 128:(j + 1) * 128]
                        for ft in range(FT):
                            base = c * (NPAD + NB) + NPAD + ft * 512 - j
                            nc.tensor.matmul(
                                yps[ft][:], lhs, xr[:, base:base + 512],
                                start=(j == 0), stop=(j == NHK - 1),
                                skip_group_check=True)
                    for ft in range(FT):
                        ysb = outp.tile([128, 512], F32, tag="ysb")
                        nc.vector.tensor_copy(ysb[:], yps[ft][:])
                        nc.sync.dma_start(
                            yt_d.ap()[:, c, ft * 512:(ft + 1) * 512], ysb[:])

    nc.compile()
    return nc


def _get_program():
    if "nc" not in _CACHE:
        _CACHE["nc"] = _build_program()
        _CACHE["consts"] = _build_constants()
    return _CACHE["nc"], _CACHE["consts"]


def _prep_core_inputs(consts, x_b, Bs_b, A1_b, A2_b):
    xr = np.zeros((C, NPAD + NB, 128), np.float32)
    xr[:, NPAD:, :] = x_b.reshape(C, NB, 128)[:, :, ::-1]
    xt = np.ascontiguousarray(xr.transpose(2, 0, 1).reshape(128, -1))
    coef = np.concatenate(
        [Bs_b[:, 0], Bs_b[:, 1], Bs_b[:, 2], A1_b, A2_b]
    ).astype(np.float32).reshape(1, 30)
    m = {"xt": xt, "coef": coef}
    m.update(consts)
    return m


def kernel(input_signal, Bs, A1_pre, A2_pre):
    from concourse import bass_utils

    nc, consts = _get_program()
    input_signal = np.asarray(input_signal, dtype=np.float32)
    Bs = np.asarray(Bs, dtype=np.float32)
    A1_pre = np.asarray(A1_pre, dtype=np.float32)
    A2_pre = np.asarray(A2_pre, dtype=np.float32)

    in_maps = [
        _prep_core_inputs(consts, input_signal[b], Bs[b], A1_pre[b], A2_pre[b])
        for b in range(B)
    ]
    res = bass_utils.run_bass_kernel_spmd(nc, in_maps, core_ids=list(range(B)))
    out = np.empty((B, C, L), np.float32)
    for b in range(B):
        yt = res.results[b]["yt"]                      # [128, C, NB]
        out[b] = yt.transpose(1, 2, 0).reshape(C, L)
    return out


# revision 5
# speedup vs baseline: 1.0258x; 1.0258x over previous
"""Trainium2 Bass kernel for nn_BiquadFilter.

Math: the reference builds, per batch, an 8192-tap FIR from 6 cascaded
biquads (frequency sampling: rfft of 3-tap coeff arrays -> cascade product
-> irfft), then linearly convolves each [C=2, L=524288] signal with it
(causal, truncated to L).

Device implementation (one batch per NeuronCore, 8 cores):
 1. tanh-activations of the feedback coefficients, broadcast to 128
    partitions via a ones-matmul.
 2. Frequency response H[f] on a [u=128, j=33] grid (f = u + 128 j) via
    DVE ops with host-provided cos/sin tables; the 6-biquad cascade is
    evaluated for all k at once on a [128, 6*33] layout using stride-0
    broadcast access patterns, then reduced by a pairwise complex
    product tree along the free dim.
 3. irfft(8192) as a 3-step factorization (contract j with a 33x128 DFT
    basis; pointwise twiddle; contract u with a 128x64 basis), giving
    fir[p + 128 q] laid out [q=64, p=128]; rounded to float32r and
    stored to a DRAM scratch with 128-zero margins.
 4. 65 Hankel-shaped stationaries hk_j[v, p] = fir[128(j-1) + 1 + p + v]
    reloaded as 5 coalesced overlapping-window DMAs (per partition v the
    (j, p) address map is linear, so each chunk is contiguous).
 5. Convolution as 2 x 65 x 8 accumulating float32r matmuls:
    y[p, 128 f] block-tiles of [128, 512] in PSUM; the input signal is
    host-relaid-out as xr[v, c, blk] = x[c, 128 blk + 127 - v] with 64
    zero pad blocks per channel (so the stationary needs only positive
    strides), and fed to the device already typed float32r.
"""

import numpy as np

FIR_LEN = 8192
L = 524288
C = 2
B = 8
K = 6
NB = L // 128            # 4096 blocks per channel
NPAD = 64                # causal zero-pad blocks
NJ = 33                  # f chunks (33*128 = 4224 >= 4097)
NQ = 64                  # fir rows (64*128 = 8192)
NHK = 65                 # conv stationaries
FT = NB // 512           # free tiles per channel (8)
XW = C * (NPAD + NB)     # xr free width (8320)

_CACHE = {}


def _build_constants():
    f = np.arange(NJ * 128)
    w = np.zeros(NJ * 128, np.float64)
    w[0] = 1.0
    w[4096] = 1.0
    w[1:4096] = 2.0
    w /= FIR_LEN
    th = 2.0 * np.pi * f / FIR_LEN
    c1 = np.cos(th)
    s1 = -np.sin(th)
    c2 = np.cos(2 * th)
    s2 = -np.sin(2 * th)
    for a in (c1, s1, c2, s2):
        a[4097:] = 0.0
    w[4097:] = 0.0

    def t(a):
        return np.ascontiguousarray(a.reshape(NJ, 128).T.astype(np.float32))

    u = np.arange(128)
    p = np.arange(128)
    j = np.arange(NJ)
    q = np.arange(NQ)
    Are = np.cos(2 * np.pi * np.outer(u, p) / FIR_LEN).astype(np.float32)
    Aim = np.sin(2 * np.pi * np.outer(u, p) / FIR_LEN).astype(np.float32)
    Bre = np.cos(2 * np.pi * np.outer(j, p) / 64).astype(np.float32)
    Bim = np.sin(2 * np.pi * np.outer(j, p) / 64).astype(np.float32)
    Cre = np.cos(2 * np.pi * np.outer(u, q) / 64).astype(np.float32)
    Cim = np.sin(2 * np.pi * np.outer(u, q) / 64).astype(np.float32)
    return {
        "c1": t(c1), "s1": t(s1), "c2": t(c2), "s2": t(s2), "wt": t(w),
        "Are": Are, "Aim": Aim,
        "Bre": np.ascontiguousarray(Bre), "Bim": np.ascontiguousarray(Bim),
        "Bimn": np.ascontiguousarray(-Bim),
        "Cre": np.ascontiguousarray(Cre), "Cimn": np.ascontiguousarray(-Cim),
        "ones": np.ones((1, 128), np.float32),
        "ident": np.eye(128, dtype=np.float32),
    }


def _build_program():
    import concourse.bass as bass
    import concourse.bacc as bacc
    import concourse.tile as tile
    from concourse import mybir

    F32 = mybir.dt.float32
    F32R = mybir.dt.float32r
    ACT = mybir.ActivationFunctionType
    MUL = mybir.AluOpType.mult
    ADD = mybir.AluOpType.add
    SUB = mybir.AluOpType.subtract

    nc = bacc.Bacc("TRN2", target_bir_lowering=False, debug=False,
                   enable_asserts=False)

    xt_d = nc.dram_tensor("xt", [128, XW], F32R, kind="ExternalInput")
    coef_d = nc.dram_tensor("coef", [1, 30], F32, kind="ExternalInput")
    tabs_d = {n: nc.dram_tensor(n, [128, NJ], F32, kind="ExternalInput")
              for n in ("c1", "s1", "c2", "s2", "wt")}
    Are_d = nc.dram_tensor("Are", [128, 128], F32, kind="ExternalInput")
    Aim_d = nc.dram_tensor("Aim", [128, 128], F32, kind="ExternalInput")
    Bre_d = nc.dram_tensor("Bre", [NJ, 128], F32, kind="ExternalInput")
    Bim_d = nc.dram_tensor("Bim", [NJ, 128], F32, kind="ExternalInput")
    Bimn_d = nc.dram_tensor("Bimn", [NJ, 128], F32, kind="ExternalInput")
    Cre_d = nc.dram_tensor("Cre", [128, NQ], F32, kind="ExternalInput")
    Cimn_d = nc.dram_tensor("Cimn", [128, NQ], F32, kind="ExternalInput")
    ones_d = nc.dram_tensor("ones", [1, 128], F32, kind="ExternalInput")
    ident_d = nc.dram_tensor("ident", [128, 128], F32, kind="ExternalInput")

    yt_d = nc.dram_tensor("yt", [128, C, NB], F32, kind="ExternalOutput")
    P_d = nc.dram_tensor("P", [FIR_LEN + 256], F32R, kind="ExternalOutput")

    def bcast(ap_t, koff, nk, nj_inner, k_is_inner):
        """AP over a [128, W] tile broadcasting to [128, nk, nj] layout."""
        pstep = ap_t.ap[0][0]
        if k_is_inner:
            # value varies along k (stride 1 from koff), bcast over j
            return bass.AP(tensor=ap_t.tensor, offset=ap_t.offset + koff,
                           ap=[[pstep, 128], [1, nk], [0, nj_inner]])
        # value varies along j, bcast over k
        return bass.AP(tensor=ap_t.tensor, offset=ap_t.offset + koff,
                       ap=[[pstep, 128], [0, nk], [1, nj_inner]])

    with tile.TileContext(nc) as tc:
        with (
            tc.tile_pool(name="const", bufs=1) as cpool,
            tc.tile_pool(name="big", bufs=1) as big,
            tc.tile_pool(name="work", bufs=2) as work,
            tc.tile_pool(name="out", bufs=3) as outp,
        ):
            # ---- big input load (already float32r-typed) ----
            xr = big.tile([128, XW], F32R)
            nc.sync.dma_start(xr[:], xt_d.ap())

            # ---- constants ----
            tabs = {}
            for n, d in tabs_d.items():
                tabs[n] = cpool.tile([128, NJ], F32, tag=n, name=n)
                nc.sync.dma_start(tabs[n][:], d.ap())
            Are = cpool.tile([128, 128], F32, tag="Are")
            nc.sync.dma_start(Are[:], Are_d.ap())
            Aim = cpool.tile([128, 128], F32, tag="Aim")
            nc.sync.dma_start(Aim[:], Aim_d.ap())
            Bre = cpool.tile([NJ, 128], F32, tag="Bre")
            nc.sync.dma_start(Bre[:], Bre_d.ap())
            Bim = cpool.tile([NJ, 128], F32, tag="Bim")
            nc.sync.dma_start(Bim[:], Bim_d.ap())
            Bimn = cpool.tile([NJ, 128], F32, tag="Bimn")
            nc.sync.dma_start(Bimn[:], Bimn_d.ap())
            Cre = cpool.tile([128, NQ], F32, tag="Cre")
            nc.sync.dma_start(Cre[:], Cre_d.ap())
            Cimn = cpool.tile([128, NQ], F32, tag="Cimn")
            nc.sync.dma_start(Cimn[:], Cimn_d.ap())
            ones = cpool.tile([1, 128], F32, tag="ones")
            nc.sync.dma_start(ones[:], ones_d.ap())
            ident = cpool.tile([128, 128], F32, tag="ident")
            nc.sync.dma_start(ident[:], ident_d.ap())

            # ---- coefficient activations on [1, n] tiles ----
            sc = cpool.tile([1, 30], F32, tag="sc")
            nc.sync.dma_start(sc[:], coef_d.ap())
            th = cpool.tile([1, 12], F32, tag="th")
            nc.scalar.activation(th[:], sc[:, 18:30], ACT.Tanh)
            scal = cpool.tile([1, 30], F32, tag="scal")
            nc.vector.tensor_copy(scal[:, 0:18], sc[:, 0:18])
            nc.scalar.mul(scal[:, 18:24], th[:, 0:6], 2.0)          # A1
            ab = cpool.tile([1, 6], F32, tag="ab")
            nc.scalar.activation(ab[:], scal[:, 18:24], ACT.Abs)    # |A1|
            tm = cpool.tile([1, 6], F32, tag="tm")
            nc.vector.tensor_mul(tm[:], ab[:], th[:, 6:12])         # |A1| t2
            x1 = cpool.tile([1, 6], F32, tag="x1")
            nc.scalar.mul(x1[:], tm[:], -0.5)
            x2 = cpool.tile([1, 6], F32, tag="x2")
            nc.scalar.mul(x2[:], ab[:], 0.5)
            x3 = cpool.tile([1, 6], F32, tag="x3")
            nc.vector.tensor_add(x3[:], th[:, 6:12], x1[:])
            nc.vector.tensor_add(scal[:, 24:30], x3[:], x2[:])      # A2

            with tc.tile_pool(name="pps", bufs=1, space="PSUM") as pps:
                # HAM warm-up: keep PE busy during the DVE prologue
                junk = pps.tile([128, 128], F32, tag="junk")
                for _ in range(22):
                    nc.tensor.matmul(junk[:], ident[:], ident[:],
                                     start=True, stop=True)

                # broadcast the 30 scalars to all partitions
                bc_ps = pps.tile([128, 30], F32, tag="bc")
                nc.tensor.matmul(bc_ps[:], ones[:], scal[:],
                                 start=True, stop=True)
                bc = cpool.tile([128, 30], F32, tag="bc_sb")
                nc.vector.tensor_copy(bc[:], bc_ps[:])

                # ---- Bf/Af for all k at once: [128, 6k, 33j] ----
                c1, s1, c2, s2 = tabs["c1"], tabs["s1"], tabs["c2"], tabs["s2"]

                def allk(basis_a, basis_b, o1, o2, extra, otag):
                    """out[u,k,j] = basis_a[u,j]*bc[u,o1+k] +
                    basis_b[u,j]*bc[u,o2+k] (+ extra)."""
                    t1 = work.tile([128, K * NJ], F32, tag="t1", name="t1")
                    v1 = t1[:].rearrange("u (k j) -> u k j", k=K)
                    nc.vector.tensor_tensor(
                        v1, bcast(c1[:], 0, K, NJ, False) if basis_a is c1
                        else bcast(basis_a[:], 0, K, NJ, False),
                        bcast(bc[:], o1, K, NJ, True), MUL)
                    t2 = work.tile([128, K * NJ], F32, tag="t2", name="t2")
                    v2 = t2[:].rearrange("u (k j) -> u k j", k=K)
                    nc.vector.tensor_tensor(
                        v2, bcast(basis_b[:], 0, K, NJ, False),
                        bcast(bc[:], o2, K, NJ, True), MUL)
                    o = work.tile([128, K * NJ], F32, tag=otag, name=otag)
                    nc.vector.tensor_add(o[:], t1[:], t2[:])
                    if extra == "b0":
                        nc.vector.tensor_tensor(
                            o[:].rearrange("u (k j) -> u k j", k=K),
                            o[:].rearrange("u (k j) -> u k j", k=K),
                            bcast(bc[:], 0, K, NJ, True), ADD)
                    elif extra == "one":
                        nc.vector.tensor_scalar_add(o[:], o[:], 1.0)
                    return o

                bfre = allk(c1, c2, 6, 12, "b0", "bfre")
                bfim = allk(s1, s2, 6, 12, None, "bfim")
                afre = allk(c1, c2, 18, 24, "one", "afre")
                afim = allk(s1, s2, 18, 24, None, "afim")

                # ---- pairwise complex product tree along k ----
                def cmul_slices(re_t, im_t, lo0, lo1, n, otag):
                    """(re,im)[:, lo0:lo0+n] * (re,im)[:, lo1:lo1+n]."""
                    w_ = n * NJ
                    a_re = re_t[:, lo0 * NJ:(lo0 + n) * NJ]
                    a_im = im_t[:, lo0 * NJ:(lo0 + n) * NJ]
                    b_re = re_t[:, lo1 * NJ:(lo1 + n) * NJ]
                    b_im = im_t[:, lo1 * NJ:(lo1 + n) * NJ]
                    t1 = work.tile([128, w_], F32, tag="ct1", name="ct1")
                    nc.vector.tensor_mul(t1[:], a_re, b_re)
                    t2 = work.tile([128, w_], F32, tag="ct2", name="ct2")
                    nc.vector.tensor_mul(t2[:], a_im, b_im)
                    orr = work.tile([128, w_], F32, tag=otag + "re",
                                    name=otag + "re")
                    nc.vector.tensor_sub(orr[:], t1[:], t2[:])
                    nc.vector.tensor_mul(t1[:], a_re, b_im)
                    nc.vector.tensor_mul(t2[:], a_im, b_re)
                    oi = work.tile([128, w_], F32, tag=otag + "im",
                                   name=otag + "im")
                    nc.vector.tensor_add(oi[:], t1[:], t2[:])
                    return orr, oi

                def cascade(re_t, im_t, otag):
                    # k: 6 -> 3 (pairs) -> product of 3
                    p3re, p3im = cmul_slices(re_t, im_t, 0, 3, 3, otag + "3")
                    q1re, q1im = cmul_slices(p3re, p3im, 0, 1, 1, otag + "q")
                    # q1 = p3[0]*p3[1]; now q1 * p3[2]
                    t1 = work.tile([128, NJ], F32, tag="ct1", name="ct1b")
                    nc.vector.tensor_mul(t1[:], q1re[:], p3re[:, 2 * NJ:3 * NJ])
                    t2 = work.tile([128, NJ], F32, tag="ct2", name="ct2b")
                    nc.vector.tensor_mul(t2[:], q1im[:], p3im[:, 2 * NJ:3 * NJ])
                    orr = work.tile([128, NJ], F32, tag=otag + "re",
                                    name=otag + "fre")
                    nc.vector.tensor_sub(orr[:], t1[:], t2[:])
                    nc.vector.tensor_mul(t1[:], q1re[:], p3im[:, 2 * NJ:3 * NJ])
                    nc.vector.tensor_mul(t2[:], q1im[:], p3re[:, 2 * NJ:3 * NJ])
                    oi = work.tile([128, NJ], F32, tag=otag + "im",
                                   name=otag + "fim")
                    nc.vector.tensor_add(oi[:], t1[:], t2[:])
                    return orr, oi

                numre, numim = cascade(bfre, bfim, "num")
                denre, denim = cascade(afre, afim, "den")

                # H = num * conj(den) / |den|^2, then * w
                d1 = work.tile([128, NJ], F32, tag="d1")
                nc.vector.tensor_mul(d1[:], denre[:], denre[:])
                d2 = work.tile([128, NJ], F32, tag="d2")
                nc.vector.tensor_mul(d2[:], denim[:], denim[:])
                dd = work.tile([128, NJ], F32, tag="dd")
                nc.vector.tensor_add(dd[:], d1[:], d2[:])
                rcp = work.tile([128, NJ], F32, tag="rcp")
                nc.vector.reciprocal(rcp[:], dd[:])
                wrcp = work.tile([128, NJ], F32, tag="wrcp")
                nc.vector.tensor_mul(wrcp[:], rcp[:], tabs["wt"][:])

                def hpart(t1in, t2in, sub, tagp):
                    t1 = work.tile([128, NJ], F32, tag="h1", name="h1")
                    nc.vector.tensor_mul(t1[:], t1in[0][:], t1in[1][:])
                    t2 = work.tile([128, NJ], F32, tag="h2", name="h2")
                    nc.vector.tensor_mul(t2[:], t2in[0][:], t2in[1][:])
                    hs = work.tile([128, NJ], F32, tag=tagp + "s",
                                   name=tagp + "s")
                    if sub:
                        nc.vector.tensor_sub(hs[:], t1[:], t2[:])
                    else:
                        nc.vector.tensor_add(hs[:], t1[:], t2[:])
                    o = work.tile([128, NJ], F32, tag=tagp, name=tagp)
                    nc.vector.tensor_mul(o[:], hs[:], wrcp[:])
                    return o

                wHre = hpart((numre, denre), (numim, denim), False, "wHre")
                wHim = hpart((numim, denre), (numre, denim), True, "wHim")

                # ---- transpose [128, 33] -> [33, 128] ----
                whreT_ps = pps.tile([NJ, 128], F32, tag="whreT")
                nc.tensor.transpose(whreT_ps[:], wHre[:], ident[:])
                whreT = work.tile([NJ, 128], F32, tag="whreTs")
                nc.vector.tensor_copy(whreT[:], whreT_ps[:])
                whimT_ps = pps.tile([NJ, 128], F32, tag="whimT")
                nc.tensor.transpose(whimT_ps[:], wHim[:], ident[:])
                whimT = work.tile([NJ, 128], F32, tag="whimTs")
                nc.vector.tensor_copy(whimT[:], whimT_ps[:])

                # ---- stage 1: T[u,p] = sum_j wH[u,j] B[j,p] ----
                tre_ps = pps.tile([128, 128], F32, tag="tre")
                nc.tensor.matmul(tre_ps[:], whreT[:], Bre[:],
                                 start=True, stop=False)
                nc.tensor.matmul(tre_ps[:], whimT[:], Bimn[:],
                                 start=False, stop=True)
                tim_ps = pps.tile([128, 128], F32, tag="tim")
                nc.tensor.matmul(tim_ps[:], whreT[:], Bim[:],
                                 start=True, stop=False)
                nc.tensor.matmul(tim_ps[:], whimT[:], Bre[:],
                                 start=False, stop=True)
                tre = work.tile([128, 128], F32, tag="tres")
                nc.vector.tensor_copy(tre[:], tre_ps[:])
                tim = work.tile([128, 128], F32, tag="tims")
                nc.vector.tensor_copy(tim[:], tim_ps[:])

                # ---- U = A (.) T ----
                u1 = work.tile([128, 128], F32, tag="u1")
                nc.vector.tensor_mul(u1[:], Are[:], tre[:])
                u2 = work.tile([128, 128], F32, tag="u2")
                nc.vector.tensor_mul(u2[:], Aim[:], tim[:])
                ure = work.tile([128, 128], F32, tag="ure")
                nc.vector.tensor_sub(ure[:], u1[:], u2[:])
                nc.vector.tensor_mul(u1[:], Are[:], tim[:])
                nc.vector.tensor_mul(u2[:], Aim[:], tre[:])
                uim = work.tile([128, 128], F32, tag="uim")
                nc.vector.tensor_add(uim[:], u1[:], u2[:])

                # ---- stage 2: fir[q,p] = sum_u Cre U_re - Cim U_im ----
                fir_ps = pps.tile([NQ, 128], F32, tag="fir")
                nc.tensor.matmul(fir_ps[:], Cre[:], ure[:],
                                 start=True, stop=False)
                nc.tensor.matmul(fir_ps[:], Cimn[:], uim[:],
                                 start=False, stop=True)
                fir_sb = work.tile([NQ, 128], F32R, tag="firs")
                nc.vector.tensor_copy(fir_sb[:], fir_ps[:])
                dst = bass.AP(tensor=P_d, offset=128, ap=[[128, NQ], [1, 128]])
                nc.sync.dma_start(dst, fir_sb[:])

            # ---- Hankel stationaries: 5 coalesced overlapping loads ----
            hk = big.tile([128, NHK * 128], F32R)
            for g, (j0, nj) in enumerate(
                    [(0, 16), (16, 16), (32, 16), (48, 16), (64, 1)]):
                src = bass.AP(tensor=P_d, offset=1 + 128 * j0,
                              ap=[[1, 128], [1, 128 * nj]])
                nc.sync.dma_start(hk[:, 128 * j0:128 * (j0 + nj)], src)

            # ---- convolution ----
            with tc.tile_pool(name="ypsum", bufs=1, space="PSUM") as yps_pool:
                for c in range(C):
                    yps = [yps_pool.tile([128, 512], F32, tag=f"y{ft}",
                                         name=f"y{ft}")
                           for ft in range(FT)]
                    for j in range(NHK):
                        lhs = hk[:, j * 128:(j + 1) * 128]
                        for ft in range(FT):
                            base = c * (NPAD + NB) + NPAD + ft * 512 - j
                            nc.tensor.matmul(
                                yps[ft][:], lhs, xr[:, base:base + 512],
                                start=(j == 0), stop=(j == NHK - 1),
                                skip_group_check=True)
                    for ft in range(FT):
                        ysb = outp.tile([128, 512], F32, tag="ysb")
                        nc.vector.tensor_copy(ysb[:], yps[ft][:])
                        nc.sync.dma_start(
                            yt_d.ap()[:, c, ft * 512:(ft + 1) * 512], ysb[:])

    nc.compile()
    return nc


def _get_program():
    if "nc" not in _CACHE:
        _CACHE["nc"] = _build_program()
        _CACHE["consts"] = _build_constants()
    return _CACHE["nc"], _CACHE["consts"]


def _prep_core_inputs(consts, x_b, Bs_b, A1_b, A2_b):
    xr = np.zeros((C, NPAD + NB, 128), np.float32)
    xr[:, NPAD:, :] = x_b.reshape(C, NB, 128)[:, :, ::-1]
    xt = np.ascontiguousarray(xr.transpose(2, 0, 1).reshape(128, -1))
    coef = np.concatenate(
        [Bs_b[:, 0], Bs_b[:, 1], Bs_b[:, 2], A1_b, A2_b]
    ).astype(np.float32).reshape(1, 30)
    m = {"xt": xt, "coef": coef}
    m.update(consts)
    return m


def kernel(input_signal, Bs, A1_pre, A2_pre):
    from concourse import bass_utils

    nc, consts = _get_program()
    input_signal = np.asarray(input_signal, dtype=np.float32)
    Bs = np.asarray(Bs, dtype=np.float32)
    A1_pre = np.asarray(A1_pre, dtype=np.float32)
    A2_pre = np.asarray(A2_pre, dtype=np.float32)

    in_maps = [
        _prep_core_inputs(consts, input_signal[b], Bs[b], A1_pre[b], A2_pre[b])
        for b in range(B)
    ]
    res = bass_utils.run_bass_kernel_spmd(nc, in_maps, core_ids=list(range(B)))
    out = np.empty((B, C, L), np.float32)
    for b in range(B):
        yt = res.results[b]["yt"]                      # [128, C, NB]
        out[b] = yt.transpose(1, 2, 0).reshape(C, L)
    return out


# revision 7
# speedup vs baseline: 1.0940x; 1.0664x over previous
"""Trainium2 Bass kernel for nn_BiquadFilter.

Math: the reference builds, per batch, an 8192-tap FIR from 6 cascaded
biquads (frequency sampling: rfft of 3-tap coeff arrays -> cascade product
-> irfft), then linearly convolves each [C=2, L=524288] signal with it
(causal, truncated to L).

Device implementation (one batch per NeuronCore, 8 cores):
 1. tanh-activations of the feedback coefficients, broadcast to 128
    partitions via a ones-matmul.
 2. Frequency response H[f] on a [u=128, j=33] grid (f = u + 128 j) via
    DVE/GpSimd ops with host-provided cos/sin tables; the 6-biquad
    cascade is evaluated for all k at once on a [128, 6*33] layout using
    stride-0 broadcast access patterns, then reduced by a pairwise
    complex product tree along the free dim.
 3. irfft(8192) as a 3-step factorization (contract j with a 33x128 DFT
    basis; pointwise twiddle; contract u with a 128x64 basis), giving
    fir[p + 128 q] laid out [q=64, p=128]; rounded to the conv dtype and
    stored to a DRAM scratch with 128-zero margins.
 4. 65 Hankel-shaped stationaries hk_j[v, p] = fir[128(j-1) + 1 + p + v]
    reloaded as 5 coalesced overlapping-window DMAs (per partition v the
    (j, p) address map is linear, so each chunk is contiguous).
 5. Convolution as 2 x 65 x 8 accumulating matmuls in the conv dtype:
    y[p, 128 f] block-tiles of [128, 512] in PSUM; the input signal is
    host-relaid-out as xr[v, c, blk] = x[c, 128 blk + 127 - v] with 64
    zero pad blocks per channel (so the stationary needs only positive
    strides), fed to the device already typed as the conv dtype.
"""

import numpy as np

FIR_LEN = 8192
L = 524288
C = 2
B = 8
K = 6
NB = L // 128            # 4096 blocks per channel
NPAD = 64                # causal zero-pad blocks
NJ = 33                  # f chunks (33*128 = 4224 >= 4097)
NQ = 64                  # fir rows (64*128 = 8192)
NHK = 65                 # conv stationaries
FT = NB // 512           # free tiles per channel (8)
XW = C * (NPAD + NB)     # xr free width (8320)

CONV_DT = "f32r"         # "f32r" | "f16"

_CACHE = {}


def _build_constants():
    f = np.arange(NJ * 128)
    w = np.zeros(NJ * 128, np.float64)
    w[0] = 1.0
    w[4096] = 1.0
    w[1:4096] = 2.0
    w /= FIR_LEN
    th = 2.0 * np.pi * f / FIR_LEN
    c1 = np.cos(th)
    s1 = -np.sin(th)
    c2 = np.cos(2 * th)
    s2 = -np.sin(2 * th)
    for a in (c1, s1, c2, s2):
        a[4097:] = 0.0
    w[4097:] = 0.0

    def t(a):
        return np.ascontiguousarray(a.reshape(NJ, 128).T.astype(np.float32))

    u = np.arange(128)
    p = np.arange(128)
    j = np.arange(NJ)
    q = np.arange(NQ)
    Are = np.cos(2 * np.pi * np.outer(u, p) / FIR_LEN).astype(np.float32)
    Aim = np.sin(2 * np.pi * np.outer(u, p) / FIR_LEN).astype(np.float32)
    Bre = np.cos(2 * np.pi * np.outer(j, p) / 64).astype(np.float32)
    Bim = np.sin(2 * np.pi * np.outer(j, p) / 64).astype(np.float32)
    Cre = np.cos(2 * np.pi * np.outer(u, q) / 64).astype(np.float32)
    Cim = np.sin(2 * np.pi * np.outer(u, q) / 64).astype(np.float32)
    return {
        "c1": t(c1), "s1": t(s1), "c2": t(c2), "s2": t(s2), "wt": t(w),
        "Are": Are, "Aim": Aim,
        "Bre": np.ascontiguousarray(Bre), "Bim": np.ascontiguousarray(Bim),
        "Bimn": np.ascontiguousarray(-Bim),
        "Cre": np.ascontiguousarray(Cre), "Cimn": np.ascontiguousarray(-Cim),
        "ones": np.ones((1, 128), np.float32),
        "ident": np.eye(128, dtype=np.float32),
    }


def _build_program():
    import concourse.bass as bass
    import concourse.bacc as bacc
    import concourse.tile as tile
    from concourse import mybir

    F32 = mybir.dt.float32
    CDT = mybir.dt.float32r if CONV_DT == "f32r" else mybir.dt.float16
    ACT = mybir.ActivationFunctionType
    MUL = mybir.AluOpType.mult
    ADD = mybir.AluOpType.add
    ABSMAX = mybir.AluOpType.abs_max

    nc = bacc.Bacc("TRN2", target_bir_lowering=False, debug=False,
                   enable_asserts=False)

    coef_d = nc.dram_tensor("coef", [1, 30], F32, kind="ExternalInput")
    xt_d = nc.dram_tensor("xt", [128, XW], CDT, kind="ExternalInput")
    tabs_d = {n: nc.dram_tensor(n, [128, NJ], F32, kind="ExternalInput")
              for n in ("c1", "s1", "c2", "s2", "wt")}
    Are_d = nc.dram_tensor("Are", [128, 128], F32, kind="ExternalInput")
    Aim_d = nc.dram_tensor("Aim", [128, 128], F32, kind="ExternalInput")
    Bre_d = nc.dram_tensor("Bre", [NJ, 128], F32, kind="ExternalInput")
    Bim_d = nc.dram_tensor("Bim", [NJ, 128], F32, kind="ExternalInput")
    Bimn_d = nc.dram_tensor("Bimn", [NJ, 128], F32, kind="ExternalInput")
    Cre_d = nc.dram_tensor("Cre", [128, NQ], F32, kind="ExternalInput")
    Cimn_d = nc.dram_tensor("Cimn", [128, NQ], F32, kind="ExternalInput")
    ones_d = nc.dram_tensor("ones", [1, 128], F32, kind="ExternalInput")
    ident_d = nc.dram_tensor("ident", [128, 128], F32, kind="ExternalInput")

    yt_d = nc.dram_tensor("yt", [128, C, NB], F32, kind="ExternalOutput")
    P_d = nc.dram_tensor("P", [FIR_LEN + 256], CDT, kind="ExternalOutput")

    def bcast(ap_t, off, nk, nj_inner, k_is_inner):
        pstep = ap_t.ap[0][0]
        if k_is_inner:
            return bass.AP(tensor=ap_t.tensor, offset=ap_t.offset + off,
                           ap=[[pstep, 128], [1, nk], [0, nj_inner]])
        return bass.AP(tensor=ap_t.tensor, offset=ap_t.offset + off,
                       ap=[[pstep, 128], [0, nk], [1, nj_inner]])

    with tile.TileContext(nc) as tc:
        with (
            tc.tile_pool(name="const", bufs=1) as cpool,
            tc.tile_pool(name="big", bufs=1) as big,
            tc.tile_pool(name="work", bufs=2) as work,
            tc.tile_pool(name="out", bufs=3) as outp,
        ):
            # ---- coefficient input FIRST (heads the DMA ring) ----
            sc = cpool.tile([1, 30], F32, tag="sc")
            nc.sync.dma_start(sc[:], coef_d.ap())

            # ---- small constants next ----
            tabs = {}
            for n, d in tabs_d.items():
                tabs[n] = cpool.tile([128, NJ], F32, tag=n, name=n)
                nc.sync.dma_start(tabs[n][:], d.ap())
            ones = cpool.tile([1, 128], F32, tag="ones")
            nc.sync.dma_start(ones[:], ones_d.ap())
            ident = cpool.tile([128, 128], F32, tag="ident")
            nc.sync.dma_start(ident[:], ident_d.ap())
            Are = cpool.tile([128, 128], F32, tag="Are")
            nc.sync.dma_start(Are[:], Are_d.ap())
            Aim = cpool.tile([128, 128], F32, tag="Aim")
            nc.sync.dma_start(Aim[:], Aim_d.ap())
            Bre = cpool.tile([NJ, 128], F32, tag="Bre")
            nc.sync.dma_start(Bre[:], Bre_d.ap())
            Bim = cpool.tile([NJ, 128], F32, tag="Bim")
            nc.sync.dma_start(Bim[:], Bim_d.ap())
            Bimn = cpool.tile([NJ, 128], F32, tag="Bimn")
            nc.sync.dma_start(Bimn[:], Bimn_d.ap())
            Cre = cpool.tile([128, NQ], F32, tag="Cre")
            nc.sync.dma_start(Cre[:], Cre_d.ap())
            Cimn = cpool.tile([128, NQ], F32, tag="Cimn")
            nc.sync.dma_start(Cimn[:], Cimn_d.ap())

            # ---- big input load LAST among initial DMAs ----
            xr = big.tile([128, XW], CDT)
            nc.sync.dma_start(xr[:], xt_d.ap())

            # ---- coefficient activations: tanh on ACT, rest on DVE ----
            th = cpool.tile([1, 12], F32, tag="th")
            nc.scalar.activation(th[:], sc[:, 18:30], ACT.Tanh)
            ab = cpool.tile([1, 6], F32, tag="ab")
            nc.scalar.activation(ab[:], th[:, 0:6], ACT.Abs)       # |tanh a1|
            scal = cpool.tile([1, 30], F32, tag="scal")
            nc.vector.tensor_copy(scal[:, 0:18], sc[:, 0:18])
            nc.vector.tensor_scalar_mul(scal[:, 18:24], th[:, 0:6], 2.0)  # A1
            # A2 = t2 + |th1| - |th1| t2   (since |A1|/2 = |th1|)
            tm = cpool.tile([1, 6], F32, tag="tm")
            nc.vector.tensor_mul(tm[:], ab[:], th[:, 6:12])
            x3 = cpool.tile([1, 6], F32, tag="x3")
            nc.vector.tensor_add(x3[:], th[:, 6:12], ab[:])
            nc.vector.tensor_sub(scal[:, 24:30], x3[:], tm[:])     # A2

            with tc.tile_pool(name="pps", bufs=1, space="PSUM") as pps:
                # HAM warm-up: keep PE busy during the DVE prologue
                junk = pps.tile([128, 128], F32, tag="junk")
                for _ in range(40):
                    nc.tensor.matmul(junk[:], ident[:], ident[:],
                                     start=True, stop=True)

                # broadcast the 30 scalars to all partitions
                bc_ps = pps.tile([128, 30], F32, tag="bc")
                nc.tensor.matmul(bc_ps[:], ones[:], scal[:],
                                 start=True, stop=True)
                bc = cpool.tile([128, 30], F32, tag="bc_sb")
                nc.vector.tensor_copy(bc[:], bc_ps[:])

                # ---- Bf/Af for all k at once: [128, 6k, 33j] ----
                # Bf on DVE, Af on GpSimd (runs in parallel)
                c1, s1, c2, s2 = tabs["c1"], tabs["s1"], tabs["c2"], tabs["s2"]

                def allk(eng, basis_a, basis_b, o1, o2, extra, otag):
                    t1 = work.tile([128, K * NJ], F32, tag=otag + "t1",
                                   name=otag + "t1")
                    eng.tensor_tensor(
                        t1[:].rearrange("u (k j) -> u k j", k=K),
                        bcast(basis_a[:], 0, K, NJ, False),
                        bcast(bc[:], o1, K, NJ, True), MUL)
                    t2 = work.tile([128, K * NJ], F32, tag=otag + "t2",
                                   name=otag + "t2")
                    eng.tensor_tensor(
                        t2[:].rearrange("u (k j) -> u k j", k=K),
                        bcast(basis_b[:], 0, K, NJ, False),
                        bcast(bc[:], o2, K, NJ, True), MUL)
                    o = work.tile([128, K * NJ], F32, tag=otag, name=otag)
                    eng.tensor_add(o[:], t1[:], t2[:])
                    if extra == "b0":
                        eng.tensor_tensor(
                            o[:].rearrange("u (k j) -> u k j", k=K),
                            o[:].rearrange("u (k j) -> u k j", k=K),
                            bcast(bc[:], 0, K, NJ, True), ADD)
                    elif extra == "one":
                        eng.tensor_scalar_add(o[:], o[:], 1.0)
                    return o

                bfre = allk(nc.vector, c1, c2, 6, 12, "b0", "bfre")
                bfim = allk(nc.vector, s1, s2, 6, 12, None, "bfim")
                afre = allk(nc.gpsimd, c1, c2, 18, 24, "one", "afre")
                afim = allk(nc.gpsimd, s1, s2, 18, 24, None, "afim")

                # ---- pairwise complex product tree along k ----
                def cmul_slices(re_t, im_t, lo0, lo1, n, otag):
                    w_ = n * NJ
                    a_re = re_t[:, lo0 * NJ:(lo0 + n) * NJ]
                    a_im = im_t[:, lo0 * NJ:(lo0 + n) * NJ]
                    b_re = re_t[:, lo1 * NJ:(lo1 + n) * NJ]
                    b_im = im_t[:, lo1 * NJ:(lo1 + n) * NJ]
                    t1 = work.tile([128, w_], F32, tag="ct1", name="ct1")
                    nc.vector.tensor_mul(t1[:], a_re, b_re)
                    t2 = work.tile([128, w_], F32, tag="ct2", name="ct2")
                    nc.vector.tensor_mul(t2[:], a_im, b_im)
                    orr = work.tile([128, w_], F32, tag=otag + "re",
                                    name=otag + "re")
                    nc.vector.tensor_sub(orr[:], t1[:], t2[:])
                    nc.vector.tensor_mul(t1[:], a_re, b_im)
                    nc.vector.tensor_mul(t2[:], a_im, b_re)
                    oi = work.tile([128, w_], F32, tag=otag + "im",
                                   name=otag + "im")
                    nc.vector.tensor_add(oi[:], t1[:], t2[:])
                    return orr, oi

                def cascade(re_t, im_t, otag):
                    p3re, p3im = cmul_slices(re_t, im_t, 0, 3, 3, otag + "3")
                    q1re, q1im = cmul_slices(p3re, p3im, 0, 1, 1, otag + "q")
                    t1 = work.tile([128, NJ], F32, tag="ct1", name="ct1b")
                    nc.vector.tensor_mul(t1[:], q1re[:], p3re[:, 2 * NJ:3 * NJ])
                    t2 = work.tile([128, NJ], F32, tag="ct2", name="ct2b")
                    nc.vector.tensor_mul(t2[:], q1im[:], p3im[:, 2 * NJ:3 * NJ])
                    orr = work.tile([128, NJ], F32, tag=otag + "re",
                                    name=otag + "fre")
                    nc.vector.tensor_sub(orr[:], t1[:], t2[:])
                    nc.vector.tensor_mul(t1[:], q1re[:], p3im[:, 2 * NJ:3 * NJ])
                    nc.vector.tensor_mul(t2[:], q1im[:], p3re[:, 2 * NJ:3 * NJ])
                    oi = work.tile([128, NJ], F32, tag=otag + "im",
                                   name=otag + "fim")
                    nc.vector.tensor_add(oi[:], t1[:], t2[:])
                    return orr, oi

                numre, numim = cascade(bfre, bfim, "num")
                denre, denim = cascade(afre, afim, "den")

                # H = num * conj(den) / |den|^2, then * w  (d on gpsimd)
                d1 = work.tile([128, NJ], F32, tag="d1")
                nc.gpsimd.tensor_mul(d1[:], denre[:], denre[:])
                d2 = work.tile([128, NJ], F32, tag="d2")
                nc.gpsimd.tensor_mul(d2[:], denim[:], denim[:])
                dd = work.tile([128, NJ], F32, tag="dd")
                nc.gpsimd.tensor_add(dd[:], d1[:], d2[:])
                rcp = work.tile([128, NJ], F32, tag="rcp")
                nc.vector.reciprocal(rcp[:], dd[:])
                wrcp = work.tile([128, NJ], F32, tag="wrcp")
                nc.vector.tensor_mul(wrcp[:], rcp[:], tabs["wt"][:])

                def hpart(t1in, t2in, sub, tagp):
                    t1 = work.tile([128, NJ], F32, tag="h1", name="h1")
                    nc.vector.tensor_mul(t1[:], t1in[0][:], t1in[1][:])
                    t2 = work.tile([128, NJ], F32, tag="h2", name="h2")
                    nc.vector.tensor_mul(t2[:], t2in[0][:], t2in[1][:])
                    hs = work.tile([128, NJ], F32, tag=tagp + "s",
                                   name=tagp + "s")
                    if sub:
                        nc.vector.tensor_sub(hs[:], t1[:], t2[:])
                    else:
                        nc.vector.tensor_add(hs[:], t1[:], t2[:])
                    o = work.tile([128, NJ], F32, tag=tagp, name=tagp)
                    nc.vector.tensor_mul(o[:], hs[:], wrcp[:])
                    return o

                wHre = hpart((numre, denre), (numim, denim), False, "wHre")
                wHim = hpart((numim, denre), (numre, denim), True, "wHim")

                # ---- transpose [128, 33] -> [33, 128] ----
                whreT_ps = pps.tile([NJ, 128], F32, tag="whreT")
                nc.tensor.transpose(whreT_ps[:], wHre[:], ident[:])
                whreT = work.tile([NJ, 128], F32, tag="whreTs")
                nc.vector.tensor_copy(whreT[:], whreT_ps[:])
                whimT_ps = pps.tile([NJ, 128], F32, tag="whimT")
                nc.tensor.transpose(whimT_ps[:], wHim[:], ident[:])
                whimT = work.tile([NJ, 128], F32, tag="whimTs")
                nc.vector.tensor_copy(whimT[:], whimT_ps[:])

                # ---- stage 1: T[u,p] = sum_j wH[u,j] B[j,p] ----
                tre_ps = pps.tile([128, 128], F32, tag="tre")
                nc.tensor.matmul(tre_ps[:], whreT[:], Bre[:],
                                 start=True, stop=False)
                nc.tensor.matmul(tre_ps[:], whimT[:], Bimn[:],
                                 start=False, stop=True)
                tim_ps = pps.tile([128, 128], F32, tag="tim")
                nc.tensor.matmul(tim_ps[:], whreT[:], Bim[:],
                                 start=True, stop=False)
                nc.tensor.matmul(tim_ps[:], whimT[:], Bre[:],
                                 start=False, stop=True)
                tre = work.tile([128, 128], F32, tag="tres")
                nc.vector.tensor_copy(tre[:], tre_ps[:])
                tim = work.tile([128, 128], F32, tag="tims")
                nc.vector.tensor_copy(tim[:], tim_ps[:])

                # ---- U = A (.) T ----
                u1 = work.tile([128, 128], F32, tag="u1")
                nc.vector.tensor_mul(u1[:], Are[:], tre[:])
                u2 = work.tile([128, 128], F32, tag="u2")
                nc.vector.tensor_mul(u2[:], Aim[:], tim[:])
                ure = work.tile([128, 128], F32, tag="ure")
                nc.vector.tensor_sub(ure[:], u1[:], u2[:])
                nc.vector.tensor_mul(u1[:], Are[:], tim[:])
                nc.vector.tensor_mul(u2[:], Aim[:], tre[:])
                uim = work.tile([128, 128], F32, tag="uim")
                nc.vector.tensor_add(uim[:], u1[:], u2[:])

                # ---- stage 2: fir[q,p] = sum_u Cre U_re - Cim U_im ----
                fir_ps = pps.tile([NQ, 128], F32, tag="fir")
                nc.tensor.matmul(fir_ps[:], Cre[:], ure[:],
                                 start=True, stop=False)
                nc.tensor.matmul(fir_ps[:], Cimn[:], uim[:],
                                 start=False, stop=True)
                fir_sb = work.tile([NQ, 128], CDT, tag="firs")
                nc.vector.tensor_copy(fir_sb[:], fir_ps[:])
                dst = bass.AP(tensor=P_d, offset=128, ap=[[128, NQ], [1, 128]])
                nc.sync.dma_start(dst, fir_sb[:])

                # keep PE warm while the Hankel reload runs
                for _ in range(14):
                    nc.tensor.matmul(junk[:], ident[:], ident[:],
                                     start=True, stop=True)

            # ---- Hankel stationaries: 5 coalesced overlapping loads ----
            hk = big.tile([128, NHK * 128], CDT)
            for j0, nj in ((0, 16), (16, 16), (32, 16), (48, 16), (64, 1)):
                src = bass.AP(tensor=P_d, offset=1 + 128 * j0,
                              ap=[[1, 128], [1, 128 * nj]])
                nc.sync.dma_start(hk[:, 128 * j0:128 * (j0 + nj)], src)

            # ---- convolution ----
            with tc.tile_pool(name="ypsum", bufs=1, space="PSUM") as yps_pool:
                for c in range(C):
                    yps = [yps_pool.tile([128, 512], mybir.dt.float32,
                                         tag=f"y{ft}", name=f"y{ft}")
                           for ft in range(FT)]
                    for j in range(NHK):
                        lhs = hk[:, j * 128:(j + 1) * 128]
                        for ft in range(FT):
                            base = c * (NPAD + NB) + NPAD + ft * 512 - j
                            nc.tensor.matmul(
                                yps[ft][:], lhs, xr[:, base:base + 512],
                                start=(j == 0), stop=(j == NHK - 1),
                                skip_group_check=True)
                    for ft in range(FT):
                        ysb = outp.tile([128, 512], mybir.dt.float32,
                                        tag="ysb")
                        nc.vector.tensor_copy(ysb[:], yps[ft][:])
                        nc.sync.dma_start(
                            yt_d.ap()[:, c, ft * 512:(ft + 1) * 512], ysb[:])

    nc.compile()
    return nc


def _get_program():
    if "nc" not in _CACHE:
        _CACHE["nc"] = _build_program()
        _CACHE["consts"] = _build_constants()
    return _CACHE["nc"], _CACHE["consts"]


def _prep_core_inputs(consts, x_b, Bs_b, A1_b, A2_b):
    np_cdt = np.float32 if CONV_DT == "f32r" else np.float16
    xr = np.zeros((C, NPAD + NB, 128), np_cdt)
    xr[:, NPAD:, :] = x_b.reshape(C, NB, 128)[:, :, ::-1]
    xt = np.ascontiguousarray(xr.transpose(2, 0, 1).reshape(128, -1))
    coef = np.concatenate(
        [Bs_b[:, 0], Bs_b[:, 1], Bs_b[:, 2], A1_b, A2_b]
    ).astype(np.float32).reshape(1, 30)
    m = {"xt": xt, "coef": coef}
    m.update(consts)
    return m


def kernel(input_signal, Bs, A1_pre, A2_pre):
    from concourse import bass_utils

    nc, consts = _get_program()
    input_signal = np.asarray(input_signal, dtype=np.float32)
    Bs = np.asarray(Bs, dtype=np.float32)
    A1_pre = np.asarray(A1_pre, dtype=np.float32)
    A2_pre = np.asarray(A2_pre, dtype=np.float32)

    in_maps = [
        _prep_core_inputs(consts, input_signal[b], Bs[b], A1_pre[b], A2_pre[b])
        for b in range(B)
    ]
    res = bass_utils.run_bass_kernel_spmd(nc, in_maps, core_ids=list(range(B)))
    out = np.empty((B, C, L), np.float32)
    for b in range(B):
        yt = res.results[b]["yt"]                      # [128, C, NB]
        out[b] = yt.transpose(1, 2, 0).reshape(C, L)
    return out


# revision 8
# speedup vs baseline: 1.1602x; 1.0606x over previous
"""Trainium2 Bass kernel for nn_BiquadFilter.

Math: the reference builds, per batch, an 8192-tap FIR from 6 cascaded
biquads (frequency sampling: rfft of 3-tap coeff arrays -> cascade product
-> irfft), then linearly convolves each [C=2, L=524288] signal with it
(causal, truncated to L).

Device implementation (one batch per NeuronCore, 8 cores):
 1. tanh-activations of the feedback coefficients, broadcast to 128
    partitions via a ones-matmul.
 2. Frequency response H[f] on a [u=128, j=33] grid (f = u + 128 j) via
    DVE/GpSimd ops with host-provided cos/sin tables; the 6-biquad
    cascade is evaluated for all k at once on a [128, 6*33] layout using
    stride-0 broadcast access patterns, then reduced by a pairwise
    complex product tree along the free dim.
 3. irfft(8192) as a 3-step factorization (contract j with a 33x128 DFT
    basis; pointwise twiddle; contract u with a 128x64 basis), giving
    fir[p + 128 q] laid out [q=64, p=128]; rounded to the conv dtype and
    stored to a DRAM scratch with 128-zero margins.
 4. 65 Hankel-shaped stationaries hk_j[v, p] = fir[128(j-1) + 1 + p + v]
    reloaded as 5 coalesced overlapping-window DMAs (per partition v the
    (j, p) address map is linear, so each chunk is contiguous).
 5. Convolution as 2 x 65 x 8 accumulating matmuls in the conv dtype:
    y[p, 128 f] block-tiles of [128, 512] in PSUM; the input signal is
    host-relaid-out as xr[v, c, blk] = x[c, 128 blk + 127 - v] with 64
    zero pad blocks per channel (so the stationary needs only positive
    strides), fed to the device already typed as the conv dtype.
"""

import numpy as np

FIR_LEN = 8192
L = 524288
C = 2
B = 8
K = 6
NB = L // 128            # 4096 blocks per channel
NPAD = 64                # causal zero-pad blocks
NJ = 33                  # f chunks (33*128 = 4224 >= 4097)
NQ = 64                  # fir rows (64*128 = 8192)
NHK = 65                 # conv stationaries
FT = NB // 512           # free tiles per channel (8)
XW = C * (NPAD + NB)     # xr free width (8320)

CONV_DT = "f16"         # "f32r" | "f16"

_CACHE = {}


def _build_constants():
    f = np.arange(NJ * 128)
    w = np.zeros(NJ * 128, np.float64)
    w[0] = 1.0
    w[4096] = 1.0
    w[1:4096] = 2.0
    w /= FIR_LEN
    th = 2.0 * np.pi * f / FIR_LEN
    c1 = np.cos(th)
    s1 = -np.sin(th)
    c2 = np.cos(2 * th)
    s2 = -np.sin(2 * th)
    for a in (c1, s1, c2, s2):
        a[4097:] = 0.0
    w[4097:] = 0.0

    def t(a):
        return np.ascontiguousarray(a.reshape(NJ, 128).T.astype(np.float32))

    u = np.arange(128)
    p = np.arange(128)
    j = np.arange(NJ)
    q = np.arange(NQ)
    Are = np.cos(2 * np.pi * np.outer(u, p) / FIR_LEN).astype(np.float32)
    Aim = np.sin(2 * np.pi * np.outer(u, p) / FIR_LEN).astype(np.float32)
    Bre = np.cos(2 * np.pi * np.outer(j, p) / 64).astype(np.float32)
    Bim = np.sin(2 * np.pi * np.outer(j, p) / 64).astype(np.float32)
    Cre = np.cos(2 * np.pi * np.outer(u, q) / 64).astype(np.float32)
    Cim = np.sin(2 * np.pi * np.outer(u, q) / 64).astype(np.float32)
    return {
        "c1": t(c1), "s1": t(s1), "c2": t(c2), "s2": t(s2), "wt": t(w),
        "Are": Are, "Aim": Aim,
        "Bre": np.ascontiguousarray(Bre), "Bim": np.ascontiguousarray(Bim),
        "Bimn": np.ascontiguousarray(-Bim),
        "Cre": np.ascontiguousarray(Cre), "Cimn": np.ascontiguousarray(-Cim),
        "ones": np.ones((1, 128), np.float32),
        "ident": np.eye(128, dtype=np.float32),
    }


def _build_program():
    import concourse.bass as bass
    import concourse.bacc as bacc
    import concourse.tile as tile
    from concourse import mybir

    F32 = mybir.dt.float32
    CDT = mybir.dt.float32r if CONV_DT == "f32r" else mybir.dt.float16
    ACT = mybir.ActivationFunctionType
    MUL = mybir.AluOpType.mult
    ADD = mybir.AluOpType.add
    ABSMAX = mybir.AluOpType.abs_max

    nc = bacc.Bacc("TRN2", target_bir_lowering=False, debug=False,
                   enable_asserts=False)

    coef_d = nc.dram_tensor("coef", [1, 30], F32, kind="ExternalInput")
    xt_d = nc.dram_tensor("xt", [128, XW], CDT, kind="ExternalInput")
    tabs_d = {n: nc.dram_tensor(n, [128, NJ], F32, kind="ExternalInput")
              for n in ("c1", "s1", "c2", "s2", "wt")}
    Are_d = nc.dram_tensor("Are", [128, 128], F32, kind="ExternalInput")
    Aim_d = nc.dram_tensor("Aim", [128, 128], F32, kind="ExternalInput")
    Bre_d = nc.dram_tensor("Bre", [NJ, 128], F32, kind="ExternalInput")
    Bim_d = nc.dram_tensor("Bim", [NJ, 128], F32, kind="ExternalInput")
    Bimn_d = nc.dram_tensor("Bimn", [NJ, 128], F32, kind="ExternalInput")
    Cre_d = nc.dram_tensor("Cre", [128, NQ], F32, kind="ExternalInput")
    Cimn_d = nc.dram_tensor("Cimn", [128, NQ], F32, kind="ExternalInput")
    ones_d = nc.dram_tensor("ones", [1, 128], F32, kind="ExternalInput")
    ident_d = nc.dram_tensor("ident", [128, 128], F32, kind="ExternalInput")

    yt_d = nc.dram_tensor("yt", [128, C, NB], F32, kind="ExternalOutput")
    P_d = nc.dram_tensor("P", [FIR_LEN + 256], CDT, kind="ExternalOutput")

    def bcast(ap_t, off, nk, nj_inner, k_is_inner):
        pstep = ap_t.ap[0][0]
        if k_is_inner:
            return bass.AP(tensor=ap_t.tensor, offset=ap_t.offset + off,
                           ap=[[pstep, 128], [1, nk], [0, nj_inner]])
        return bass.AP(tensor=ap_t.tensor, offset=ap_t.offset + off,
                       ap=[[pstep, 128], [0, nk], [1, nj_inner]])

    with tile.TileContext(nc) as tc:
        with (
            tc.tile_pool(name="const", bufs=1) as cpool,
            tc.tile_pool(name="big", bufs=1) as big,
            tc.tile_pool(name="work", bufs=2) as work,
            tc.tile_pool(name="out", bufs=3) as outp,
        ):
            # ---- coefficient input FIRST (heads the DMA ring) ----
            sc = cpool.tile([1, 30], F32, tag="sc")
            nc.sync.dma_start(sc[:], coef_d.ap())

            # ---- small constants next ----
            tabs = {}
            for n, d in tabs_d.items():
                tabs[n] = cpool.tile([128, NJ], F32, tag=n, name=n)
                nc.sync.dma_start(tabs[n][:], d.ap())
            ones = cpool.tile([1, 128], F32, tag="ones")
            nc.sync.dma_start(ones[:], ones_d.ap())
            ident = cpool.tile([128, 128], F32, tag="ident")
            nc.sync.dma_start(ident[:], ident_d.ap())
            Are = cpool.tile([128, 128], F32, tag="Are")
            nc.sync.dma_start(Are[:], Are_d.ap())
            Aim = cpool.tile([128, 128], F32, tag="Aim")
            nc.sync.dma_start(Aim[:], Aim_d.ap())
            Bre = cpool.tile([NJ, 128], F32, tag="Bre")
            nc.sync.dma_start(Bre[:], Bre_d.ap())
            Bim = cpool.tile([NJ, 128], F32, tag="Bim")
            nc.sync.dma_start(Bim[:], Bim_d.ap())
            Bimn = cpool.tile([NJ, 128], F32, tag="Bimn")
            nc.sync.dma_start(Bimn[:], Bimn_d.ap())
            Cre = cpool.tile([128, NQ], F32, tag="Cre")
            nc.sync.dma_start(Cre[:], Cre_d.ap())
            Cimn = cpool.tile([128, NQ], F32, tag="Cimn")
            nc.sync.dma_start(Cimn[:], Cimn_d.ap())

            # ---- big input load LAST among initial DMAs ----
            xr = big.tile([128, XW], CDT)
            nc.sync.dma_start(xr[:], xt_d.ap())

            # ---- coefficient activations: tanh on ACT, rest on DVE ----
            th = cpool.tile([1, 12], F32, tag="th")
            nc.scalar.activation(th[:], sc[:, 18:30], ACT.Tanh)
            ab = cpool.tile([1, 6], F32, tag="ab")
            nc.scalar.activation(ab[:], th[:, 0:6], ACT.Abs)       # |tanh a1|
            scal = cpool.tile([1, 30], F32, tag="scal")
            nc.vector.tensor_copy(scal[:, 0:18], sc[:, 0:18])
            nc.vector.tensor_scalar_mul(scal[:, 18:24], th[:, 0:6], 2.0)  # A1
            # A2 = t2 + |th1| - |th1| t2   (since |A1|/2 = |th1|)
            tm = cpool.tile([1, 6], F32, tag="tm")
            nc.vector.tensor_mul(tm[:], ab[:], th[:, 6:12])
            x3 = cpool.tile([1, 6], F32, tag="x3")
            nc.vector.tensor_add(x3[:], th[:, 6:12], ab[:])
            nc.vector.tensor_sub(scal[:, 24:30], x3[:], tm[:])     # A2

            with tc.tile_pool(name="pps", bufs=1, space="PSUM") as pps:
                # HAM warm-up: keep PE busy during the DVE prologue
                junk = pps.tile([128, 128], F32, tag="junk")
                for _ in range(40):
                    nc.tensor.matmul(junk[:], ident[:], ident[:],
                                     start=True, stop=True)

                # broadcast the 30 scalars to all partitions
                bc_ps = pps.tile([128, 30], F32, tag="bc")
                nc.tensor.matmul(bc_ps[:], ones[:], scal[:],
                                 start=True, stop=True)
                bc = cpool.tile([128, 30], F32, tag="bc_sb")
                nc.vector.tensor_copy(bc[:], bc_ps[:])

                # ---- Bf/Af for all k at once: [128, 6k, 33j] ----
                # Bf on DVE, Af on GpSimd (runs in parallel)
                c1, s1, c2, s2 = tabs["c1"], tabs["s1"], tabs["c2"], tabs["s2"]

                def allk(eng, basis_a, basis_b, o1, o2, extra, otag):
                    t1 = work.tile([128, K * NJ], F32, tag=otag + "t1",
                                   name=otag + "t1")
                    eng.tensor_tensor(
                        t1[:].rearrange("u (k j) -> u k j", k=K),
                        bcast(basis_a[:], 0, K, NJ, False),
                        bcast(bc[:], o1, K, NJ, True), MUL)
                    t2 = work.tile([128, K * NJ], F32, tag=otag + "t2",
                                   name=otag + "t2")
                    eng.tensor_tensor(
                        t2[:].rearrange("u (k j) -> u k j", k=K),
                        bcast(basis_b[:], 0, K, NJ, False),
                        bcast(bc[:], o2, K, NJ, True), MUL)
                    o = work.tile([128, K * NJ], F32, tag=otag, name=otag)
                    eng.tensor_add(o[:], t1[:], t2[:])
                    if extra == "b0":
                        eng.tensor_tensor(
                            o[:].rearrange("u (k j) -> u k j", k=K),
                            o[:].rearrange("u (k j) -> u k j", k=K),
                            bcast(bc[:], 0, K, NJ, True), ADD)
                    elif extra == "one":
                        eng.tensor_scalar_add(o[:], o[:], 1.0)
                    return o

                bfre = allk(nc.vector, c1, c2, 6, 12, "b0", "bfre")
                bfim = allk(nc.vector, s1, s2, 6, 12, None, "bfim")
                afre = allk(nc.gpsimd, c1, c2, 18, 24, "one", "afre")
                afim = allk(nc.gpsimd, s1, s2, 18, 24, None, "afim")

                # ---- pairwise complex product tree along k ----
                def cmul_slices(re_t, im_t, lo0, lo1, n, otag):
                    w_ = n * NJ
                    a_re = re_t[:, lo0 * NJ:(lo0 + n) * NJ]
                    a_im = im_t[:, lo0 * NJ:(lo0 + n) * NJ]
                    b_re = re_t[:, lo1 * NJ:(lo1 + n) * NJ]
                    b_im = im_t[:, lo1 * NJ:(lo1 + n) * NJ]
                    t1 = work.tile([128, w_], F32, tag="ct1", name="ct1")
                    nc.vector.tensor_mul(t1[:], a_re, b_re)
                    t2 = work.tile([128, w_], F32, tag="ct2", name="ct2")
                    nc.vector.tensor_mul(t2[:], a_im, b_im)
                    orr = work.tile([128, w_], F32, tag=otag + "re",
                                    name=otag + "re")
                    nc.vector.tensor_sub(orr[:], t1[:], t2[:])
                    nc.vector.tensor_mul(t1[:], a_re, b_im)
                    nc.vector.tensor_mul(t2[:], a_im, b_re)
                    oi = work.tile([128, w_], F32, tag=otag + "im",
                                   name=otag + "im")
                    nc.vector.tensor_add(oi[:], t1[:], t2[:])
                    return orr, oi

                def cascade(re_t, im_t, otag):
                    p3re, p3im = cmul_slices(re_t, im_t, 0, 3, 3, otag + "3")
                    q1re, q1im = cmul_slices(p3re, p3im, 0, 1, 1, otag + "q")
                    t1 = work.tile([128, NJ], F32, tag="ct1", name="ct1b")
                    nc.vector.tensor_mul(t1[:], q1re[:], p3re[:, 2 * NJ:3 * NJ])
                    t2 = work.tile([128, NJ], F32, tag="ct2", name="ct2b")
                    nc.vector.tensor_mul(t2[:], q1im[:], p3im[:, 2 * NJ:3 * NJ])
                    orr = work.tile([128, NJ], F32, tag=otag + "re",
                                    name=otag + "fre")
                    nc.vector.tensor_sub(orr[:], t1[:], t2[:])
                    nc.vector.tensor_mul(t1[:], q1re[:], p3im[:, 2 * NJ:3 * NJ])
                    nc.vector.tensor_mul(t2[:], q1im[:], p3re[:, 2 * NJ:3 * NJ])
                    oi = work.tile([128, NJ], F32, tag=otag + "im",
                                   name=otag + "fim")
                    nc.vector.tensor_add(oi[:], t1[:], t2[:])
                    return orr, oi

                numre, numim = cascade(bfre, bfim, "num")
                denre, denim = cascade(afre, afim, "den")

                # H = num * conj(den) / |den|^2, then * w  (d on gpsimd)
                d1 = work.tile([128, NJ], F32, tag="d1")
                nc.gpsimd.tensor_mul(d1[:], denre[:], denre[:])
                d2 = work.tile([128, NJ], F32, tag="d2")
                nc.gpsimd.tensor_mul(d2[:], denim[:], denim[:])
                dd = work.tile([128, NJ], F32, tag="dd")
                nc.gpsimd.tensor_add(dd[:], d1[:], d2[:])
                rcp = work.tile([128, NJ], F32, tag="rcp")
                nc.vector.reciprocal(rcp[:], dd[:])
                wrcp = work.tile([128, NJ], F32, tag="wrcp")
                nc.vector.tensor_mul(wrcp[:], rcp[:], tabs["wt"][:])

                def hpart(t1in, t2in, sub, tagp):
                    t1 = work.tile([128, NJ], F32, tag="h1", name="h1")
                    nc.vector.tensor_mul(t1[:], t1in[0][:], t1in[1][:])
                    t2 = work.tile([128, NJ], F32, tag="h2", name="h2")
                    nc.vector.tensor_mul(t2[:], t2in[0][:], t2in[1][:])
                    hs = work.tile([128, NJ], F32, tag=tagp + "s",
                                   name=tagp + "s")
                    if sub:
                        nc.vector.tensor_sub(hs[:], t1[:], t2[:])
                    else:
                        nc.vector.tensor_add(hs[:], t1[:], t2[:])
                    o = work.tile([128, NJ], F32, tag=tagp, name=tagp)
                    nc.vector.tensor_mul(o[:], hs[:], wrcp[:])
                    return o

                wHre = hpart((numre, denre), (numim, denim), False, "wHre")
                wHim = hpart((numim, denre), (numre, denim), True, "wHim")

                # ---- transpose [128, 33] -> [33, 128] ----
                whreT_ps = pps.tile([NJ, 128], F32, tag="whreT")
                nc.tensor.transpose(whreT_ps[:], wHre[:], ident[:])
                whreT = work.tile([NJ, 128], F32, tag="whreTs")
                nc.vector.tensor_copy(whreT[:], whreT_ps[:])
                whimT_ps = pps.tile([NJ, 128], F32, tag="whimT")
                nc.tensor.transpose(whimT_ps[:], wHim[:], ident[:])
                whimT = work.tile([NJ, 128], F32, tag="whimTs")
                nc.vector.tensor_copy(whimT[:], whimT_ps[:])

                # ---- stage 1: T[u,p] = sum_j wH[u,j] B[j,p] ----
                tre_ps = pps.tile([128, 128], F32, tag="tre")
                nc.tensor.matmul(tre_ps[:], whreT[:], Bre[:],
                                 start=True, stop=False)
                nc.tensor.matmul(tre_ps[:], whimT[:], Bimn[:],
                                 start=False, stop=True)
                tim_ps = pps.tile([128, 128], F32, tag="tim")
                nc.tensor.matmul(tim_ps[:], whreT[:], Bim[:],
                                 start=True, stop=False)
                nc.tensor.matmul(tim_ps[:], whimT[:], Bre[:],
                                 start=False, stop=True)
                tre = work.tile([128, 128], F32, tag="tres")
                nc.vector.tensor_copy(tre[:], tre_ps[:])
                tim = work.tile([128, 128], F32, tag="tims")
                nc.vector.tensor_copy(tim[:], tim_ps[:])

                # ---- U = A (.) T ----
                u1 = work.tile([128, 128], F32, tag="u1")
                nc.vector.tensor_mul(u1[:], Are[:], tre[:])
                u2 = work.tile([128, 128], F32, tag="u2")
                nc.vector.tensor_mul(u2[:], Aim[:], tim[:])
                ure = work.tile([128, 128], F32, tag="ure")
                nc.vector.tensor_sub(ure[:], u1[:], u2[:])
                nc.vector.tensor_mul(u1[:], Are[:], tim[:])
                nc.vector.tensor_mul(u2[:], Aim[:], tre[:])
                uim = work.tile([128, 128], F32, tag="uim")
                nc.vector.tensor_add(uim[:], u1[:], u2[:])

                # ---- stage 2: fir[q,p] = sum_u Cre U_re - Cim U_im ----
                fir_ps = pps.tile([NQ, 128], F32, tag="fir")
                nc.tensor.matmul(fir_ps[:], Cre[:], ure[:],
                                 start=True, stop=False)
                nc.tensor.matmul(fir_ps[:], Cimn[:], uim[:],
                                 start=False, stop=True)
                fir_sb = work.tile([NQ, 128], CDT, tag="firs")
                nc.vector.tensor_copy(fir_sb[:], fir_ps[:])
                dst = bass.AP(tensor=P_d, offset=128, ap=[[128, NQ], [1, 128]])
                nc.sync.dma_start(dst, fir_sb[:])

                # keep PE warm while the Hankel reload runs
                for _ in range(14):
                    nc.tensor.matmul(junk[:], ident[:], ident[:],
                                     start=True, stop=True)

            # ---- Hankel stationaries: 5 coalesced overlapping loads ----
            hk = big.tile([128, NHK * 128], CDT)
            for j0, nj in ((0, 16), (16, 16), (32, 16), (48, 16), (64, 1)):
                src = bass.AP(tensor=P_d, offset=1 + 128 * j0,
                              ap=[[1, 128], [1, 128 * nj]])
                nc.sync.dma_start(hk[:, 128 * j0:128 * (j0 + nj)], src)

            # ---- convolution ----
            with tc.tile_pool(name="ypsum", bufs=1, space="PSUM") as yps_pool:
                for c in range(C):
                    yps = [yps_pool.tile([128, 512], mybir.dt.float32,
                                         tag=f"y{ft}", name=f"y{ft}")
                           for ft in range(FT)]
                    for j in range(NHK):
                        lhs = hk[:, j * 128:(j + 1) * 128]
                        for ft in range(FT):
                            base = c * (NPAD + NB) + NPAD + ft * 512 - j
                            nc.tensor.matmul(
                                yps[ft][:], lhs, xr[:, base:base + 512],
                                start=(j == 0), stop=(j == NHK - 1),
                                skip_group_check=True)
                    for ft in range(FT):
                        ysb = outp.tile([128, 512], mybir.dt.float32,
                                        tag="ysb")
                        nc.vector.tensor_copy(ysb[:], yps[ft][:])
                        nc.sync.dma_start(
                            yt_d.ap()[:, c, ft * 512:(ft + 1) * 512], ysb[:])

    nc.compile()
    return nc


def _get_program():
    if "nc" not in _CACHE:
        _CACHE["nc"] = _build_program()
        _CACHE["consts"] = _build_constants()
    return _CACHE["nc"], _CACHE["consts"]


def _prep_core_inputs(consts, x_b, Bs_b, A1_b, A2_b):
    np_cdt = np.float32 if CONV_DT == "f32r" else np.float16
    xr = np.zeros((C, NPAD + NB, 128), np_cdt)
    xr[:, NPAD:, :] = x_b.reshape(C, NB, 128)[:, :, ::-1]
    xt = np.ascontiguousarray(xr.transpose(2, 0, 1).reshape(128, -1))
    coef = np.concatenate(
        [Bs_b[:, 0], Bs_b[:, 1], Bs_b[:, 2], A1_b, A2_b]
    ).astype(np.float32).reshape(1, 30)
    m = {"xt": xt, "coef": coef}
    m.update(consts)
    return m


def kernel(input_signal, Bs, A1_pre, A2_pre):
    from concourse import bass_utils

    nc, consts = _get_program()
    input_signal = np.asarray(input_signal, dtype=np.float32)
    Bs = np.asarray(Bs, dtype=np.float32)
    A1_pre = np.asarray(A1_pre, dtype=np.float32)
    A2_pre = np.asarray(A2_pre, dtype=np.float32)

    in_maps = [
        _prep_core_inputs(consts, input_signal[b], Bs[b], A1_pre[b], A2_pre[b])
        for b in range(B)
    ]
    res = bass_utils.run_bass_kernel_spmd(nc, in_maps, core_ids=list(range(B)))
    out = np.empty((B, C, L), np.float32)
    for b in range(B):
        yt = res.results[b]["yt"]                      # [128, C, NB]
        out[b] = yt.transpose(1, 2, 0).reshape(C, L)
    return out
